# revision 28
# baseline (speedup 1.0000x reference)
"""KNN (k=16, 10 classes) on 8 Trainium2 NeuronCores via Bass/Tile.

Strategy (classic distributed ANN):
  - Host: sort X_train by label; shard N_train contiguously across 8 cores;
    pad each per-core class segment to a 512 multiple so every 512-wide
    matmul chunk is class-pure (label == per-chunk constant).
  - Device (per core, SPMD): v[q, j] = 2<t_q, x_j> - ||x_j||^2 computed by
    PE as accumulating matmuls into PSUM [128q x 512n].  Ordering by v ==
    ordering by -euclidean distance.  DVE max8 reads each PSUM chunk
    directly -> per-chunk top-8 values, written straight to the output
    candidate buffer (no on-device stage 2).
  - Host: merge 8 cores x C*8 candidates per query by value; candidate
    slot -> chunk -> label (chunks are class-pure), majority vote.

Matmul dtype: float32r (1 cycle/row at N=512 vs 4 for float32).  If
PRECISE=True, uses an exact-ish bf16 hi/lo decomposition instead:
cross = th.xh + th.xl + tl.xh (tl.xl dropped, ~2^-18 relative) and the
-||x||^2 term via a K=2 ones matmul against [nxx_hi; nxx_lo].
"""

import numpy as np

NCORES = 8
CHUNK = 512
K = 16
NUM_CLASSES = 10
QTILE = 128
PRECISE = True  # False: float32r matmuls; True: bf16 hi/lo 3-pass

_compiled_cache = {}


def _patch_ldw_opt():
    # The staged toolchain config disables walrus's weight-load optimizer,
    # which forces a serialized ~95ns LDWEIGHTS before every matmul.
    # Re-enable it for our NEFF (applies at neuronx-cc time).
    try:
        from concourse import compiler_utils as cu
        flags = cu.get_compiler_flags()
        nf = [f.replace("--enable-ldw-opt=false", "--enable-ldw-opt=true")
              for f in flags]
        if nf != flags:
            cu.set_compiler_flags(nf)
    except Exception:
        pass


def _build_program(D, NQ, Lp, precise):
    import concourse.bacc as bacc
    import concourse.tile as tile
    import concourse.mybir as mybir

    C = Lp // CHUNK
    nqt = NQ // QTILE
    f32 = mybir.dt.float32
    f32r = mybir.dt.float32r
    bf16 = mybir.dt.bfloat16

    nc = bacc.Bacc("TRN2", target_bir_lowering=False, debug=False)
    # hi/lo bf16 planes of 2*X_test.T and X_train.T; [3,Lp] hi/mid/lo -xx
    xth = nc.dram_tensor("xth", [D, NQ], bf16, kind="ExternalInput")
    xtl = nc.dram_tensor("xtl", [D, NQ], bf16, kind="ExternalInput")
    xnh = nc.dram_tensor("xnh", [D, Lp], bf16, kind="ExternalInput")
    xnl = nc.dram_tensor("xnl", [D, Lp], bf16, kind="ExternalInput")
    nxx2 = nc.dram_tensor("nxx2", [3, Lp], bf16, kind="ExternalInput")
    ones = nc.dram_tensor("ones", [3, QTILE], bf16, kind="ExternalInput")
    u32 = mybir.dt.uint32
    out_val = nc.dram_tensor("out_val", [NQ, K], f32, kind="ExternalOutput")
    out_idx = nc.dram_tensor("out_idx", [NQ, K], u32, kind="ExternalOutput")

    with tile.TileContext(nc) as tc:
        with tc.tile_pool(name="res", bufs=1) as res, \
             tc.tile_pool(name="psum", bufs=1, space="PSUM") as ps, \
             tc.tile_pool(name="cand", bufs=3) as candp:
            # Per-chunk train tiles so the first matmuls don't wait on the
            # whole X_train DMA; issue order: what qtile-0/chunk-0 needs
            # first, then interleave the rest.
            if precise:
                ones_t = res.tile([3, QTILE], bf16)
                nc.sync.dma_start(out=ones_t[:], in_=ones[:])
                xth_t = [res.tile([D, QTILE], bf16, name=f"xth{q}",
                                  tag=f"xth{q}") for q in range(nqt)]
                xtl_t = [res.tile([D, QTILE], bf16, name=f"xtl{q}",
                                  tag=f"xtl{q}") for q in range(nqt)]
                xnh_t = [res.tile([D, CHUNK], bf16, name=f"xnh{c}",
                                  tag=f"xnh{c}") for c in range(C)]
                xnl_t = [res.tile([D, CHUNK], bf16, name=f"xnl{c}",
                                  tag=f"xnl{c}") for c in range(C)]
                nxx_t = [res.tile([3, CHUNK], bf16, name=f"nxx{c}",
                                  tag=f"nxx{c}") for c in range(C)]
                qsl0 = slice(0, QTILE)
                nc.sync.dma_start(out=xth_t[0][:], in_=xth[:, qsl0])
                nc.sync.dma_start(out=xtl_t[0][:], in_=xtl[:, qsl0])
                for ci in range(C):
                    sl = slice(ci * CHUNK, (ci + 1) * CHUNK)
                    nc.sync.dma_start(out=xnh_t[ci][:], in_=xnh[:, sl])
                    nc.sync.dma_start(out=xnl_t[ci][:], in_=xnl[:, sl])
                    nc.sync.dma_start(out=nxx_t[ci][:], in_=nxx2[:, sl])
                for qt in range(1, nqt):
                    sl = slice(qt * QTILE, (qt + 1) * QTILE)
                    nc.sync.dma_start(out=xth_t[qt][:], in_=xth[:, sl])
                    nc.sync.dma_start(out=xtl_t[qt][:], in_=xtl[:, sl])
            # HAM warmup: dense dummy matmuls on the first tile to land
            # (ones, 768B) so the PE clock is at 8/8 before real work.
            warm = ps.tile([QTILE, QTILE], f32, name="warm", tag="ps0")
            for _ in range(64):
                nc.tensor.matmul(warm[:], lhsT=ones_t[:],
                                 rhs=ones_t[:], start=True, stop=True)

            GROUP = 8
            with tc.tile_pool(name="st2", bufs=2) as st2:
              def emit_stage2(qt, cand):
                # per-core top-16 (values + candidate slots)
                m1 = st2.tile([QTILE, 8], f32, tag="m1", name="m1")
                i1 = st2.tile([QTILE, 8], u32, tag="i1", name="i1")
                cand2 = candp.tile([QTILE, C * 8], f32, tag="cand2",
                                   name="cand2")
                m2 = st2.tile([QTILE, 8], f32, tag="m2", name="m2")
                i2 = st2.tile([QTILE, 8], u32, tag="i2", name="i2")
                nc.vector.max(out=m1[:], in_=cand[:])
                nc.vector.max_index(out=i1[:], in_max=m1[:], in_values=cand[:])
                nc.vector.match_replace(out=cand2[:], in_to_replace=m1[:],
                                        in_values=cand[:], imm_value=-3e38)
                nc.vector.max(out=m2[:], in_=cand2[:])
                nc.vector.max_index(out=i2[:], in_max=m2[:],
                                    in_values=cand2[:])
                vout = st2.tile([QTILE, K], f32, tag="vout", name="vout")
                iout = st2.tile([QTILE, K], u32, tag="iout", name="iout")
                nc.vector.tensor_copy(vout[:, 0:8], m1[:])
                nc.vector.tensor_copy(vout[:, 8:16], m2[:])
                nc.vector.tensor_copy(iout[:, 0:8], i1[:])
                nc.vector.tensor_copy(iout[:, 8:16], i2[:])
                qsl = slice(qt * QTILE, (qt + 1) * QTILE)
                nc.sync.dma_start(out=out_val[qsl], in_=vout[:])
                nc.sync.dma_start(out=out_idx[qsl], in_=iout[:])

              pending = []  # (qt, cand) awaiting stage 2

              # Prologue: first J qtiles chunk-outer, so each arriving
              # X_train chunk immediately feeds J qtiles of matmul work
              # (the input DMA stream is slower than one qtile's compute).
              J = 4
              pcands = {qt: candp.tile([QTILE, C * 8], f32,
                                       name=f"candp{qt}", tag=f"candp{qt}")
                        for qt in range(J)}
              for c in range(C):
                  psums = {qt: ps.tile([QTILE, CHUNK], f32,
                                       name=f"ps{(c * J + qt) % 8}",
                                       tag=f"ps{(c * J + qt) % 8}")
                           for qt in range(J)}
                  for qt in range(J):
                      nc.tensor.matmul(psums[qt][:], lhsT=xth_t[qt][:],
                                       rhs=xnh_t[c][:],
                                       start=True, stop=False)
                      nc.tensor.matmul(psums[qt][:], lhsT=xth_t[qt][:],
                                       rhs=xnl_t[c][:],
                                       start=False, stop=False)
                  for qt in range(J):
                      nc.tensor.matmul(psums[qt][:], lhsT=xtl_t[qt][:],
                                       rhs=xnh_t[c][:],
                                       start=False, stop=False)
                  for qt in range(J):
                      nc.tensor.matmul(psums[qt][:], lhsT=ones_t[:],
                                       rhs=nxx_t[c][:],
                                       start=False, stop=True)
                  for qt in range(J):
                      nc.vector.max(out=pcands[qt][:, c * 8:(c + 1) * 8],
                                    in_=psums[qt][:])
              pending.extend((qt, pcands[qt]) for qt in range(J))

              for qt in range(J, nqt):
                cand = candp.tile([QTILE, C * 8], f32, name="cand",
                                  tag="cand")
                for g in range(0, C, GROUP):
                    cs = list(range(g, min(g + GROUP, C)))
                    psums = {c: ps.tile([QTILE, CHUNK], f32,
                                        name=f"ps{c % 8}", tag=f"ps{c % 8}")
                             for c in cs}
                    # phase-major: same stationary weights back-to-back
                    for c in cs:
                        nc.tensor.matmul(psums[c][:], lhsT=xth_t[qt][:],
                                         rhs=xnh_t[c][:],
                                         start=True, stop=False)
                        nc.tensor.matmul(psums[c][:], lhsT=xth_t[qt][:],
                                         rhs=xnl_t[c][:],
                                         start=False, stop=False)
                    for c in cs:
                        nc.tensor.matmul(psums[c][:], lhsT=xtl_t[qt][:],
                                         rhs=xnh_t[c][:],
                                         start=False, stop=False)
                    for c in cs:
                        nc.tensor.matmul(psums[c][:], lhsT=ones_t[:],
                                         rhs=nxx_t[c][:],
                                         start=False, stop=True)
                    for c in cs:
                        nc.vector.max(out=cand[:, c * 8:(c + 1) * 8],
                                      in_=psums[c][:])
                    if g == 0 and pending:
                        emit_stage2(*pending.pop(0))
                pending.append((qt, cand))
              for p in pending:
                  emit_stage2(*p)

    nc.compile()
    return nc


def _get_program(D, NQ, Lp, precise):
    key = (D, NQ, Lp, precise)
    if key not in _compiled_cache:
        _compiled_cache[key] = _build_program(D, NQ, Lp, precise)
    return _compiled_cache[key]


def _bf16_split(a):
    import ml_dtypes
    hi = a.astype(ml_dtypes.bfloat16)
    lo = (a - hi.astype(np.float32)).astype(ml_dtypes.bfloat16)
    return hi, lo


def prepare(X_train, y_train, X_test):
    """Host prep: shard/sort/pad; returns (nc, in_maps, aux)."""
    X_train = np.ascontiguousarray(np.asarray(X_train, dtype=np.float32))
    X_test = np.ascontiguousarray(np.asarray(X_test, dtype=np.float32))
    y_np = np.asarray(y_train)
    N, D = X_train.shape
    NQ = X_test.shape[0]

    # ---- host prep: label-sort, pad each class to a CHUNK multiple
    # globally, then split the global chunk sequence evenly across cores
    # (chunk-aligned => class-pure chunks, minimal max per-core length).
    order = np.argsort(y_np, kind="stable")
    Xs = X_train[order]
    ys = y_np[order]
    xx = np.einsum("ij,ij->i", Xs.astype(np.float64), Xs.astype(np.float64))
    xx = xx.astype(np.float32)

    b = [0] + list(np.nonzero(np.diff(ys))[0] + 1) + [N]
    segs = [(b[i], b[i + 1], int(ys[b[i]])) for i in range(len(b) - 1)]
    T = sum((e - s + CHUNK - 1) // CHUNK for s, e, _ in segs)
    C = (T + NCORES - 1) // NCORES          # chunks per core
    Lp = C * CHUNK

    PAD_XX = np.float32(4e9)
    gX = np.zeros((D, NCORES * C * CHUNK), np.float32)
    gnxx = np.full((1, NCORES * C * CHUNK), -PAD_XX, np.float32)
    glab = np.zeros(NCORES * C, np.int64)
    pos = 0
    for s, e, lab in segs:
        n = e - s
        gX[:, pos:pos + n] = Xs[s:e].T
        gnxx[0, pos:pos + n] = -xx[s:e]
        nch = (n + CHUNK - 1) // CHUNK
        glab[pos // CHUNK:pos // CHUNK + nch] = lab
        pos += nch * CHUNK

    xnT = np.ascontiguousarray(
        gX.reshape(D, NCORES, Lp).swapaxes(0, 1))
    nxx = np.ascontiguousarray(
        gnxx.reshape(1, NCORES, Lp).swapaxes(0, 1))
    chunk_label = glab.reshape(NCORES, C)

    xtT = np.ascontiguousarray((2.0 * X_test).T)  # [D, NQ], exact x2

    _patch_ldw_opt()
    nc = _get_program(D, NQ, Lp, PRECISE)
    if PRECISE:
        import ml_dtypes
        xth, xtl = _bf16_split(xtT)
        ones_np = np.ones((3, QTILE), ml_dtypes.bfloat16)
        in_maps = []
        for k in range(NCORES):
            xnh, xnl = _bf16_split(xnT[k])
            nxh, nxm = _bf16_split(nxx[k])
            nxl = (nxx[k] - nxh.astype(np.float32)
                   - nxm.astype(np.float32)).astype(ml_dtypes.bfloat16)
            in_maps.append({
                "xth": np.ascontiguousarray(xth),
                "xtl": np.ascontiguousarray(xtl),
                "xnh": np.ascontiguousarray(xnh),
                "xnl": np.ascontiguousarray(xnl),
                "nxx2": np.ascontiguousarray(
                    np.concatenate([nxh, nxm, nxl], axis=0)),
                "ones": ones_np,
            })
    else:
        ones_np = np.ones((1, QTILE), np.float32)
        in_maps = [{"xt": xtT, "xn": np.ascontiguousarray(xnT[k]),
                    "nxx": nxx[k], "ones": ones_np} for k in range(NCORES)]
    return nc, in_maps, (chunk_label, NQ, C)


def merge(results, aux):
    """Host merge: 8 cores x 16 candidates/query -> global top-16 -> vote."""
    chunk_label, NQ, C = aux
    vals = np.stack([results[k]["out_val"] for k in range(NCORES)], axis=1)
    idxs = np.stack([results[k]["out_idx"] for k in range(NCORES)], axis=1)
    vals = vals.reshape(NQ, NCORES * K)
    labs = chunk_label[
        np.repeat(np.arange(NCORES)[None, :], NQ, axis=0).repeat(K, axis=1),
        (idxs.reshape(NQ, NCORES * K).astype(np.int64) >> 3)]
    sel = np.argpartition(-vals, K - 1, axis=1)[:, :K]
    top_lab = np.take_along_axis(labs, sel, axis=1)
    counts = np.zeros((NQ, NUM_CLASSES), np.int64)
    for c in range(NUM_CLASSES):
        counts[:, c] = (top_lab == c).sum(1)
    return counts.argmax(1).astype(np.int64)


def kernel(X_train, y_train, X_test):
    from concourse.bass_utils import run_bass_kernel_spmd
    nc, in_maps, aux = prepare(X_train, y_train, X_test)
    res = run_bass_kernel_spmd(nc, in_maps, core_ids=list(range(NCORES)))
    return merge(res.results, aux)


# revision 29
# speedup vs baseline: 1.0132x; 1.0132x over previous
"""KNN (k=16, 10 classes) on 8 Trainium2 NeuronCores via Bass/Tile.

Strategy (classic distributed ANN):
  - Host: sort X_train by label; shard N_train contiguously across 8 cores;
    pad each per-core class segment to a 512 multiple so every 512-wide
    matmul chunk is class-pure (label == per-chunk constant).
  - Device (per core, SPMD): v[q, j] = 2<t_q, x_j> - ||x_j||^2 computed by
    PE as accumulating matmuls into PSUM [128q x 512n].  Ordering by v ==
    ordering by -euclidean distance.  DVE max8 reads each PSUM chunk
    directly -> per-chunk top-8 values, written straight to the output
    candidate buffer (no on-device stage 2).
  - Host: merge 8 cores x C*8 candidates per query by value; candidate
    slot -> chunk -> label (chunks are class-pure), majority vote.

Matmul dtype: float32r (1 cycle/row at N=512 vs 4 for float32).  If
PRECISE=True, uses an exact-ish bf16 hi/lo decomposition instead:
cross = th.xh + th.xl + tl.xh (tl.xl dropped, ~2^-18 relative) and the
-||x||^2 term via a K=2 ones matmul against [nxx_hi; nxx_lo].
"""

import numpy as np

NCORES = 8
CHUNK = 512
K = 16
NUM_CLASSES = 10
QTILE = 128
PRECISE = True  # False: float32r matmuls; True: bf16 hi/lo 3-pass

_compiled_cache = {}


def _patch_ldw_opt():
    # The staged toolchain config disables walrus's weight-load optimizer,
    # which forces a serialized ~95ns LDWEIGHTS before every matmul.
    # Re-enable it for our NEFF (applies at neuronx-cc time).
    try:
        from concourse import compiler_utils as cu
        flags = cu.get_compiler_flags()
        nf = [f.replace("--enable-ldw-opt=false", "--enable-ldw-opt=true")
              for f in flags]
        if nf != flags:
            cu.set_compiler_flags(nf)
    except Exception:
        pass


def _build_program(D, NQ, Lp, precise):
    import concourse.bacc as bacc
    import concourse.tile as tile
    import concourse.mybir as mybir

    C = Lp // CHUNK
    nqt = NQ // QTILE
    f32 = mybir.dt.float32
    f32r = mybir.dt.float32r
    bf16 = mybir.dt.bfloat16

    nc = bacc.Bacc("TRN2", target_bir_lowering=False, debug=False)
    # hi/lo bf16 planes of 2*X_test.T and X_train.T; [3,Lp] hi/mid/lo -xx
    xth = nc.dram_tensor("xth", [D, NQ], bf16, kind="ExternalInput")
    xtl = nc.dram_tensor("xtl", [D, NQ], bf16, kind="ExternalInput")
    xnh = nc.dram_tensor("xnh", [D, Lp], bf16, kind="ExternalInput")
    xnl = nc.dram_tensor("xnl", [D, Lp], bf16, kind="ExternalInput")
    nxx2 = nc.dram_tensor("nxx2", [3, Lp], bf16, kind="ExternalInput")
    ones = nc.dram_tensor("ones", [3, QTILE], bf16, kind="ExternalInput")
    u32 = mybir.dt.uint32
    out_val = nc.dram_tensor("out_val", [NQ, K], f32, kind="ExternalOutput")
    out_idx = nc.dram_tensor("out_idx", [NQ, K], u32, kind="ExternalOutput")

    with tile.TileContext(nc) as tc:
        with tc.tile_pool(name="res", bufs=1) as res, \
             tc.tile_pool(name="psum", bufs=1, space="PSUM") as ps, \
             tc.tile_pool(name="cand", bufs=3) as candp:
            # Per-chunk train tiles so the first matmuls don't wait on the
            # whole X_train DMA; issue order: what qtile-0/chunk-0 needs
            # first, then interleave the rest.
            if precise:
                ones_t = res.tile([3, QTILE], bf16)
                nc.sync.dma_start(out=ones_t[:], in_=ones[:])
                xth_t = [res.tile([D, QTILE], bf16, name=f"xth{q}",
                                  tag=f"xth{q}") for q in range(nqt)]
                xtl_t = [res.tile([D, QTILE], bf16, name=f"xtl{q}",
                                  tag=f"xtl{q}") for q in range(nqt)]
                # chunk-pair tiles: 2 chunks per DMA = 2KB/partition per
                # transfer (1KB lines run at ~half DMA throughput)
                NP = (C + 1) // 2
                W2 = 2 * CHUNK
                xnhp = [res.tile([D, W2], bf16, name=f"xnhp{p}",
                                 tag=f"xnhp{p}") for p in range(NP)]
                xnlp = [res.tile([D, W2], bf16, name=f"xnlp{p}",
                                 tag=f"xnlp{p}") for p in range(NP)]
                nxxp = [res.tile([3, W2], bf16, name=f"nxxp{p}",
                                 tag=f"nxxp{p}") for p in range(NP)]
                xnh_t = [xnhp[c // 2][:, (c % 2) * CHUNK:(c % 2 + 1) * CHUNK]
                         for c in range(C)]
                xnl_t = [xnlp[c // 2][:, (c % 2) * CHUNK:(c % 2 + 1) * CHUNK]
                         for c in range(C)]
                nxx_t = [nxxp[c // 2][:, (c % 2) * CHUNK:(c % 2 + 1) * CHUNK]
                         for c in range(C)]
                qsl0 = slice(0, QTILE)
                nc.sync.dma_start(out=xth_t[0][:], in_=xth[:, qsl0])
                nc.sync.dma_start(out=xtl_t[0][:], in_=xtl[:, qsl0])
                for p in range(NP):
                    w = min(W2, Lp - p * W2)
                    sl = slice(p * W2, p * W2 + w)
                    nc.sync.dma_start(out=xnhp[p][:, 0:w], in_=xnh[:, sl])
                    nc.sync.dma_start(out=xnlp[p][:, 0:w], in_=xnl[:, sl])
                    nc.sync.dma_start(out=nxxp[p][:, 0:w], in_=nxx2[:, sl])
                for qt in range(1, nqt):
                    sl = slice(qt * QTILE, (qt + 1) * QTILE)
                    nc.sync.dma_start(out=xth_t[qt][:], in_=xth[:, sl])
                    nc.sync.dma_start(out=xtl_t[qt][:], in_=xtl[:, sl])
            # HAM warmup: dense dummy matmuls on the first tile to land
            # (ones, 768B) so the PE clock is at 8/8 before real work.
            warm = ps.tile([QTILE, QTILE], f32, name="warm", tag="ps0")
            for _ in range(64):
                nc.tensor.matmul(warm[:], lhsT=ones_t[:],
                                 rhs=ones_t[:], start=True, stop=True)

            GROUP = 8
            with tc.tile_pool(name="st2", bufs=2) as st2:
              def emit_stage2(qt, cand):
                # per-core top-16 (values + candidate slots)
                m1 = st2.tile([QTILE, 8], f32, tag="m1", name="m1")
                i1 = st2.tile([QTILE, 8], u32, tag="i1", name="i1")
                cand2 = candp.tile([QTILE, C * 8], f32, tag="cand2",
                                   name="cand2")
                m2 = st2.tile([QTILE, 8], f32, tag="m2", name="m2")
                i2 = st2.tile([QTILE, 8], u32, tag="i2", name="i2")
                nc.vector.max(out=m1[:], in_=cand[:])
                nc.vector.max_index(out=i1[:], in_max=m1[:], in_values=cand[:])
                nc.vector.match_replace(out=cand2[:], in_to_replace=m1[:],
                                        in_values=cand[:], imm_value=-3e38)
                nc.vector.max(out=m2[:], in_=cand2[:])
                nc.vector.max_index(out=i2[:], in_max=m2[:],
                                    in_values=cand2[:])
                vout = st2.tile([QTILE, K], f32, tag="vout", name="vout")
                iout = st2.tile([QTILE, K], u32, tag="iout", name="iout")
                nc.vector.tensor_copy(vout[:, 0:8], m1[:])
                nc.vector.tensor_copy(vout[:, 8:16], m2[:])
                nc.vector.tensor_copy(iout[:, 0:8], i1[:])
                nc.vector.tensor_copy(iout[:, 8:16], i2[:])
                qsl = slice(qt * QTILE, (qt + 1) * QTILE)
                nc.sync.dma_start(out=out_val[qsl], in_=vout[:])
                nc.sync.dma_start(out=out_idx[qsl], in_=iout[:])

              pending = []  # (qt, cand) awaiting stage 2

              # Prologue: first J qtiles chunk-outer, so each arriving
              # X_train chunk immediately feeds J qtiles of matmul work
              # (the input DMA stream is slower than one qtile's compute).
              J = 4
              pcands = {qt: candp.tile([QTILE, C * 8], f32,
                                       name=f"candp{qt}", tag=f"candp{qt}")
                        for qt in range(J)}
              for c in range(C):
                  psums = {qt: ps.tile([QTILE, CHUNK], f32,
                                       name=f"ps{(c * J + qt) % 8}",
                                       tag=f"ps{(c * J + qt) % 8}")
                           for qt in range(J)}
                  for qt in range(J):
                      nc.tensor.matmul(psums[qt][:], lhsT=xth_t[qt][:],
                                       rhs=xnh_t[c][:],
                                       start=True, stop=False)
                      nc.tensor.matmul(psums[qt][:], lhsT=xth_t[qt][:],
                                       rhs=xnl_t[c][:],
                                       start=False, stop=False)
                  for qt in range(J):
                      nc.tensor.matmul(psums[qt][:], lhsT=xtl_t[qt][:],
                                       rhs=xnh_t[c][:],
                                       start=False, stop=False)
                  for qt in range(J):
                      nc.tensor.matmul(psums[qt][:], lhsT=ones_t[:],
                                       rhs=nxx_t[c][:],
                                       start=False, stop=True)
                  for qt in range(J):
                      nc.vector.max(out=pcands[qt][:, c * 8:(c + 1) * 8],
                                    in_=psums[qt][:])
              pending.extend((qt, pcands[qt]) for qt in range(J))

              for qt in range(J, nqt):
                cand = candp.tile([QTILE, C * 8], f32, name="cand",
                                  tag="cand")
                for g in range(0, C, GROUP):
                    cs = list(range(g, min(g + GROUP, C)))
                    psums = {c: ps.tile([QTILE, CHUNK], f32,
                                        name=f"ps{c % 8}", tag=f"ps{c % 8}")
                             for c in cs}
                    # phase-major: same stationary weights back-to-back
                    for c in cs:
                        nc.tensor.matmul(psums[c][:], lhsT=xth_t[qt][:],
                                         rhs=xnh_t[c][:],
                                         start=True, stop=False)
                        nc.tensor.matmul(psums[c][:], lhsT=xth_t[qt][:],
                                         rhs=xnl_t[c][:],
                                         start=False, stop=False)
                    for c in cs:
                        nc.tensor.matmul(psums[c][:], lhsT=xtl_t[qt][:],
                                         rhs=xnh_t[c][:],
                                         start=False, stop=False)
                    for c in cs:
                        nc.tensor.matmul(psums[c][:], lhsT=ones_t[:],
                                         rhs=nxx_t[c][:],
                                         start=False, stop=True)
                    for c in cs:
                        nc.vector.max(out=cand[:, c * 8:(c + 1) * 8],
                                      in_=psums[c][:])
                    if g == 0 and pending:
                        emit_stage2(*pending.pop(0))
                pending.append((qt, cand))
              for p in pending:
                  emit_stage2(*p)

    nc.compile()
    return nc


def _get_program(D, NQ, Lp, precise):
    key = (D, NQ, Lp, precise)
    if key not in _compiled_cache:
        _compiled_cache[key] = _build_program(D, NQ, Lp, precise)
    return _compiled_cache[key]


def _bf16_split(a):
    import ml_dtypes
    hi = a.astype(ml_dtypes.bfloat16)
    lo = (a - hi.astype(np.float32)).astype(ml_dtypes.bfloat16)
    return hi, lo


def prepare(X_train, y_train, X_test):
    """Host prep: shard/sort/pad; returns (nc, in_maps, aux)."""
    X_train = np.ascontiguousarray(np.asarray(X_train, dtype=np.float32))
    X_test = np.ascontiguousarray(np.asarray(X_test, dtype=np.float32))
    y_np = np.asarray(y_train)
    N, D = X_train.shape
    NQ = X_test.shape[0]

    # ---- host prep: label-sort, pad each class to a CHUNK multiple
    # globally, then split the global chunk sequence evenly across cores
    # (chunk-aligned => class-pure chunks, minimal max per-core length).
    order = np.argsort(y_np, kind="stable")
    Xs = X_train[order]
    ys = y_np[order]
    xx = np.einsum("ij,ij->i", Xs.astype(np.float64), Xs.astype(np.float64))
    xx = xx.astype(np.float32)

    b = [0] + list(np.nonzero(np.diff(ys))[0] + 1) + [N]
    segs = [(b[i], b[i + 1], int(ys[b[i]])) for i in range(len(b) - 1)]
    T = sum((e - s + CHUNK - 1) // CHUNK for s, e, _ in segs)
    C = (T + NCORES - 1) // NCORES          # chunks per core
    Lp = C * CHUNK

    PAD_XX = np.float32(4e9)
    gX = np.zeros((D, NCORES * C * CHUNK), np.float32)
    gnxx = np.full((1, NCORES * C * CHUNK), -PAD_XX, np.float32)
    glab = np.zeros(NCORES * C, np.int64)
    pos = 0
    for s, e, lab in segs:
        n = e - s
        gX[:, pos:pos + n] = Xs[s:e].T
        gnxx[0, pos:pos + n] = -xx[s:e]
        nch = (n + CHUNK - 1) // CHUNK
        glab[pos // CHUNK:pos // CHUNK + nch] = lab
        pos += nch * CHUNK

    xnT = np.ascontiguousarray(
        gX.reshape(D, NCORES, Lp).swapaxes(0, 1))
    nxx = np.ascontiguousarray(
        gnxx.reshape(1, NCORES, Lp).swapaxes(0, 1))
    chunk_label = glab.reshape(NCORES, C)

    xtT = np.ascontiguousarray((2.0 * X_test).T)  # [D, NQ], exact x2

    _patch_ldw_opt()
    nc = _get_program(D, NQ, Lp, PRECISE)
    if PRECISE:
        import ml_dtypes
        xth, xtl = _bf16_split(xtT)
        ones_np = np.ones((3, QTILE), ml_dtypes.bfloat16)
        in_maps = []
        for k in range(NCORES):
            xnh, xnl = _bf16_split(xnT[k])
            nxh, nxm = _bf16_split(nxx[k])
            nxl = (nxx[k] - nxh.astype(np.float32)
                   - nxm.astype(np.float32)).astype(ml_dtypes.bfloat16)
            in_maps.append({
                "xth": np.ascontiguousarray(xth),
                "xtl": np.ascontiguousarray(xtl),
                "xnh": np.ascontiguousarray(xnh),
                "xnl": np.ascontiguousarray(xnl),
                "nxx2": np.ascontiguousarray(
                    np.concatenate([nxh, nxm, nxl], axis=0)),
                "ones": ones_np,
            })
    else:
        ones_np = np.ones((1, QTILE), np.float32)
        in_maps = [{"xt": xtT, "xn": np.ascontiguousarray(xnT[k]),
                    "nxx": nxx[k], "ones": ones_np} for k in range(NCORES)]
    return nc, in_maps, (chunk_label, NQ, C)


def merge(results, aux):
    """Host merge: 8 cores x 16 candidates/query -> global top-16 -> vote."""
    chunk_label, NQ, C = aux
    vals = np.stack([results[k]["out_val"] for k in range(NCORES)], axis=1)
    idxs = np.stack([results[k]["out_idx"] for k in range(NCORES)], axis=1)
    vals = vals.reshape(NQ, NCORES * K)
    labs = chunk_label[
        np.repeat(np.arange(NCORES)[None, :], NQ, axis=0).repeat(K, axis=1),
        (idxs.reshape(NQ, NCORES * K).astype(np.int64) >> 3)]
    sel = np.argpartition(-vals, K - 1, axis=1)[:, :K]
    top_lab = np.take_along_axis(labs, sel, axis=1)
    counts = np.zeros((NQ, NUM_CLASSES), np.int64)
    for c in range(NUM_CLASSES):
        counts[:, c] = (top_lab == c).sum(1)
    return counts.argmax(1).astype(np.int64)


def kernel(X_train, y_train, X_test):
    from concourse.bass_utils import run_bass_kernel_spmd
    nc, in_maps, aux = prepare(X_train, y_train, X_test)
    res = run_bass_kernel_spmd(nc, in_maps, core_ids=list(range(NCORES)))
    return merge(res.results, aux)


# revision 34
# speedup vs baseline: 1.0241x; 1.0108x over previous
"""KNN (k=16, 10 classes) on 8 Trainium2 NeuronCores via Bass/Tile.

Strategy (classic distributed ANN):
  - Host: sort X_train by label; shard N_train contiguously across 8 cores;
    pad each per-core class segment to a 512 multiple so every 512-wide
    matmul chunk is class-pure (label == per-chunk constant).
  - Device (per core, SPMD): v[q, j] = 2<t_q, x_j> - ||x_j||^2 computed by
    PE as accumulating matmuls into PSUM [128q x 512n].  Ordering by v ==
    ordering by -euclidean distance.  DVE max8 reads each PSUM chunk
    directly -> per-chunk top-8 values, written straight to the output
    candidate buffer (no on-device stage 2).
  - Host: merge 8 cores x C*8 candidates per query by value; candidate
    slot -> chunk -> label (chunks are class-pure), majority vote.

Matmul dtype: float32r (1 cycle/row at N=512 vs 4 for float32).  If
PRECISE=True, uses an exact-ish bf16 hi/lo decomposition instead:
cross = th.xh + th.xl + tl.xh (tl.xl dropped, ~2^-18 relative) and the
-||x||^2 term via a K=2 ones matmul against [nxx_hi; nxx_lo].
"""

import numpy as np

NCORES = 8
CHUNK = 512
K = 16
NUM_CLASSES = 10
QTILE = 128
PRECISE = True  # False: float32r matmuls; True: bf16 hi/lo 3-pass

_compiled_cache = {}


def _build_program(D, NQ, Lp, precise):
    import concourse.bacc as bacc
    import concourse.tile as tile
    import concourse.mybir as mybir

    C = Lp // CHUNK
    nqt = NQ // QTILE
    f32 = mybir.dt.float32
    f32r = mybir.dt.float32r
    bf16 = mybir.dt.bfloat16

    nc = bacc.Bacc("TRN2", target_bir_lowering=False, debug=False)
    # hi/lo bf16 planes of 2*X_test.T and X_train.T; [3,Lp] hi/mid/lo -xx
    xth = nc.dram_tensor("xth", [D, NQ], bf16, kind="ExternalInput")
    xtl = nc.dram_tensor("xtl", [D, NQ], bf16, kind="ExternalInput")
    xnh = nc.dram_tensor("xnh", [D, Lp], bf16, kind="ExternalInput")
    xnl = nc.dram_tensor("xnl", [D, Lp], bf16, kind="ExternalInput")
    nxx2 = nc.dram_tensor("nxx2", [3, Lp], bf16, kind="ExternalInput")
    ones = nc.dram_tensor("ones", [3, QTILE], bf16, kind="ExternalInput")
    u32 = mybir.dt.uint32
    out_val = nc.dram_tensor("out_val", [NQ, K], f32, kind="ExternalOutput")
    out_idx = nc.dram_tensor("out_idx", [NQ, K], u32, kind="ExternalOutput")

    with tile.TileContext(nc) as tc:
        with tc.tile_pool(name="res", bufs=1) as res, \
             tc.tile_pool(name="psum", bufs=1, space="PSUM") as ps, \
             tc.tile_pool(name="cand", bufs=4) as candp:
            # Per-chunk train tiles so the first matmuls don't wait on the
            # whole X_train DMA; issue order: what qtile-0/chunk-0 needs
            # first, then interleave the rest.
            if precise:
                ones_t = res.tile([3, QTILE], bf16)
                nc.sync.dma_start(out=ones_t[:], in_=ones[:])
                xth_t = [res.tile([D, QTILE], bf16, name=f"xth{q}",
                                  tag=f"xth{q}") for q in range(nqt)]
                xtl_t = [res.tile([D, QTILE], bf16, name=f"xtl{q}",
                                  tag=f"xtl{q}") for q in range(nqt)]
                # chunk-quad tiles: 4 chunks per DMA = 4KB/partition per
                # transfer (1KB lines run at ~half DMA throughput)
                NP = (C + 3) // 4
                W2 = 4 * CHUNK
                xnhp = [res.tile([D, W2], bf16, name=f"xnhp{p}",
                                 tag=f"xnhp{p}") for p in range(NP)]
                xnlp = [res.tile([D, W2], bf16, name=f"xnlp{p}",
                                 tag=f"xnlp{p}") for p in range(NP)]
                nxxp = [res.tile([3, W2], bf16, name=f"nxxp{p}",
                                 tag=f"nxxp{p}") for p in range(NP)]
                xnh_t = [xnhp[c // 4][:, (c % 4) * CHUNK:(c % 4 + 1) * CHUNK]
                         for c in range(C)]
                xnl_t = [xnlp[c // 4][:, (c % 4) * CHUNK:(c % 4 + 1) * CHUNK]
                         for c in range(C)]
                nxx_t = [nxxp[c // 4][:, (c % 4) * CHUNK:(c % 4 + 1) * CHUNK]
                         for c in range(C)]
                qsl0 = slice(0, QTILE)
                nc.sync.dma_start(out=xth_t[0][:], in_=xth[:, qsl0])
                nc.sync.dma_start(out=xtl_t[0][:], in_=xtl[:, qsl0])
                for p in range(NP):
                    w = min(W2, Lp - p * W2)
                    sl = slice(p * W2, p * W2 + w)
                    nc.sync.dma_start(out=xnhp[p][:, 0:w], in_=xnh[:, sl])
                    nc.sync.dma_start(out=xnlp[p][:, 0:w], in_=xnl[:, sl])
                    nc.sync.dma_start(out=nxxp[p][:, 0:w], in_=nxx2[:, sl])
                for qt in range(1, nqt):
                    sl = slice(qt * QTILE, (qt + 1) * QTILE)
                    nc.sync.dma_start(out=xth_t[qt][:], in_=xth[:, sl])
                    nc.sync.dma_start(out=xtl_t[qt][:], in_=xtl[:, sl])
            # HAM warmup: dense dummy matmuls on the first tile to land
            # (ones, 768B) so the PE clock is at 8/8 before real work.
            warm = ps.tile([QTILE, QTILE], f32, name="warm", tag="ps0")
            for _ in range(64):
                nc.tensor.matmul(warm[:], lhsT=ones_t[:],
                                 rhs=ones_t[:], start=True, stop=True)

            GROUP = 8
            with tc.tile_pool(name="st2", bufs=2) as st2:
              def emit_stage2(qt, cand):
                # per-core top-16 (values + candidate slots)
                m1 = st2.tile([QTILE, 8], f32, tag="m1", name="m1")
                i1 = st2.tile([QTILE, 8], u32, tag="i1", name="i1")
                cand2 = candp.tile([QTILE, C * 8], f32, tag="cand2",
                                   name="cand2")
                m2 = st2.tile([QTILE, 8], f32, tag="m2", name="m2")
                i2 = st2.tile([QTILE, 8], u32, tag="i2", name="i2")
                nc.vector.max(out=m1[:], in_=cand[:])
                nc.vector.max_index(out=i1[:], in_max=m1[:], in_values=cand[:])
                nc.vector.match_replace(out=cand2[:], in_to_replace=m1[:],
                                        in_values=cand[:], imm_value=-3e38)
                nc.vector.max(out=m2[:], in_=cand2[:])
                nc.vector.max_index(out=i2[:], in_max=m2[:],
                                    in_values=cand2[:])
                vout = st2.tile([QTILE, K], f32, tag="vout", name="vout")
                iout = st2.tile([QTILE, K], u32, tag="iout", name="iout")
                nc.vector.tensor_copy(vout[:, 0:8], m1[:])
                nc.vector.tensor_copy(vout[:, 8:16], m2[:])
                nc.vector.tensor_copy(iout[:, 0:8], i1[:])
                nc.vector.tensor_copy(iout[:, 8:16], i2[:])
                qsl = slice(qt * QTILE, (qt + 1) * QTILE)
                nc.sync.dma_start(out=out_val[qsl], in_=vout[:])
                nc.sync.dma_start(out=out_idx[qsl], in_=iout[:])

              pending = []  # (qt, cand) awaiting stage 2

              # Prologue: first J qtiles chunk-outer, so each arriving
              # X_train chunk immediately feeds J qtiles of matmul work
              # (the input DMA stream is slower than one qtile's compute).
              J = 4
              pcands = {qt: candp.tile([QTILE, C * 8], f32,
                                       name=f"candp{qt}", tag=f"candp{qt}")
                        for qt in range(J)}
              for c in range(C):
                  psums = {qt: ps.tile([QTILE, CHUNK], f32,
                                       name=f"ps{(c * J + qt) % 8}",
                                       tag=f"ps{(c * J + qt) % 8}")
                           for qt in range(J)}
                  for qt in range(J):
                      nc.tensor.matmul(psums[qt][:], lhsT=xth_t[qt][:],
                                       rhs=xnh_t[c][:],
                                       start=True, stop=False)
                      nc.tensor.matmul(psums[qt][:], lhsT=xth_t[qt][:],
                                       rhs=xnl_t[c][:],
                                       start=False, stop=False)
                  for qt in range(J):
                      nc.tensor.matmul(psums[qt][:], lhsT=xtl_t[qt][:],
                                       rhs=xnh_t[c][:],
                                       start=False, stop=False)
                  for qt in range(J):
                      nc.tensor.matmul(psums[qt][:], lhsT=ones_t[:],
                                       rhs=nxx_t[c][:],
                                       start=False, stop=True)
                  for qt in range(J):
                      nc.vector.max(out=pcands[qt][:, c * 8:(c + 1) * 8],
                                    in_=psums[qt][:])
              pending.extend((qt, pcands[qt]) for qt in range(J))

              for qt in range(J, nqt):
                cand = candp.tile([QTILE, C * 8], f32, name="cand",
                                  tag="cand")
                for g in range(0, C, GROUP):
                    cs = list(range(g, min(g + GROUP, C)))
                    psums = {c: ps.tile([QTILE, CHUNK], f32,
                                        name=f"ps{c % 8}", tag=f"ps{c % 8}")
                             for c in cs}
                    # phase-major: same stationary weights back-to-back
                    for c in cs:
                        nc.tensor.matmul(psums[c][:], lhsT=xth_t[qt][:],
                                         rhs=xnh_t[c][:],
                                         start=True, stop=False)
                        nc.tensor.matmul(psums[c][:], lhsT=xth_t[qt][:],
                                         rhs=xnl_t[c][:],
                                         start=False, stop=False)
                    for c in cs:
                        nc.tensor.matmul(psums[c][:], lhsT=xtl_t[qt][:],
                                         rhs=xnh_t[c][:],
                                         start=False, stop=False)
                    for c in cs:
                        nc.tensor.matmul(psums[c][:], lhsT=ones_t[:],
                                         rhs=nxx_t[c][:],
                                         start=False, stop=True)
                    for c in cs:
                        nc.vector.max(out=cand[:, c * 8:(c + 1) * 8],
                                      in_=psums[c][:])
                    if g == 0 and pending:
                        emit_stage2(*pending.pop(0))
                pending.append((qt, cand))
              for p in pending:
                  emit_stage2(*p)

    nc.compile()
    return nc


def _get_program(D, NQ, Lp, precise):
    key = (D, NQ, Lp, precise)
    if key not in _compiled_cache:
        _compiled_cache[key] = _build_program(D, NQ, Lp, precise)
    return _compiled_cache[key]


def _bf16_split(a):
    import ml_dtypes
    hi = a.astype(ml_dtypes.bfloat16)
    lo = (a - hi.astype(np.float32)).astype(ml_dtypes.bfloat16)
    return hi, lo


def prepare(X_train, y_train, X_test):
    """Host prep: shard/sort/pad; returns (nc, in_maps, aux)."""
    X_train = np.ascontiguousarray(np.asarray(X_train, dtype=np.float32))
    X_test = np.ascontiguousarray(np.asarray(X_test, dtype=np.float32))
    y_np = np.asarray(y_train)
    N, D = X_train.shape
    NQ = X_test.shape[0]

    # ---- host prep: label-sort, pad each class to a CHUNK multiple
    # globally, then split the global chunk sequence evenly across cores
    # (chunk-aligned => class-pure chunks, minimal max per-core length).
    order = np.argsort(y_np, kind="stable")
    Xs = X_train[order]
    ys = y_np[order]
    xx = np.einsum("ij,ij->i", Xs.astype(np.float64), Xs.astype(np.float64))
    xx = xx.astype(np.float32)

    b = [0] + list(np.nonzero(np.diff(ys))[0] + 1) + [N]
    segs = [(b[i], b[i + 1], int(ys[b[i]])) for i in range(len(b) - 1)]
    T = sum((e - s + CHUNK - 1) // CHUNK for s, e, _ in segs)
    C = (T + NCORES - 1) // NCORES          # chunks per core
    Lp = C * CHUNK

    PAD_XX = np.float32(4e9)
    gX = np.zeros((D, NCORES * C * CHUNK), np.float32)
    gnxx = np.full((1, NCORES * C * CHUNK), -PAD_XX, np.float32)
    glab = np.zeros(NCORES * C, np.int64)
    pos = 0
    for s, e, lab in segs:
        n = e - s
        gX[:, pos:pos + n] = Xs[s:e].T
        gnxx[0, pos:pos + n] = -xx[s:e]
        nch = (n + CHUNK - 1) // CHUNK
        glab[pos // CHUNK:pos // CHUNK + nch] = lab
        pos += nch * CHUNK

    xnT = np.ascontiguousarray(
        gX.reshape(D, NCORES, Lp).swapaxes(0, 1))
    nxx = np.ascontiguousarray(
        gnxx.reshape(1, NCORES, Lp).swapaxes(0, 1))
    chunk_label = glab.reshape(NCORES, C)

    xtT = np.ascontiguousarray((2.0 * X_test).T)  # [D, NQ], exact x2

    nc = _get_program(D, NQ, Lp, PRECISE)
    if PRECISE:
        import ml_dtypes
        xth, xtl = _bf16_split(xtT)
        ones_np = np.ones((3, QTILE), ml_dtypes.bfloat16)
        in_maps = []
        for k in range(NCORES):
            xnh, xnl = _bf16_split(xnT[k])
            nxh, nxm = _bf16_split(nxx[k])
            nxl = (nxx[k] - nxh.astype(np.float32)
                   - nxm.astype(np.float32)).astype(ml_dtypes.bfloat16)
            in_maps.append({
                "xth": np.ascontiguousarray(xth),
                "xtl": np.ascontiguousarray(xtl),
                "xnh": np.ascontiguousarray(xnh),
                "xnl": np.ascontiguousarray(xnl),
                "nxx2": np.ascontiguousarray(
                    np.concatenate([nxh, nxm, nxl], axis=0)),
                "ones": ones_np,
            })
    else:
        ones_np = np.ones((1, QTILE), np.float32)
        in_maps = [{"xt": xtT, "xn": np.ascontiguousarray(xnT[k]),
                    "nxx": nxx[k], "ones": ones_np} for k in range(NCORES)]
    return nc, in_maps, (chunk_label, NQ, C)


def merge(results, aux):
    """Host merge: 8 cores x 16 candidates/query -> global top-16 -> vote."""
    chunk_label, NQ, C = aux
    vals = np.stack([results[k]["out_val"] for k in range(NCORES)], axis=1)
    idxs = np.stack([results[k]["out_idx"] for k in range(NCORES)], axis=1)
    vals = vals.reshape(NQ, NCORES * K)
    labs = chunk_label[
        np.repeat(np.arange(NCORES)[None, :], NQ, axis=0).repeat(K, axis=1),
        (idxs.reshape(NQ, NCORES * K).astype(np.int64) >> 3)]
    sel = np.argpartition(-vals, K - 1, axis=1)[:, :K]
    top_lab = np.take_along_axis(labs, sel, axis=1)
    counts = np.zeros((NQ, NUM_CLASSES), np.int64)
    for c in range(NUM_CLASSES):
        counts[:, c] = (top_lab == c).sum(1)
    return counts.argmax(1).astype(np.int64)


def kernel(X_train, y_train, X_test):
    from concourse.bass_utils import run_bass_kernel_spmd
    nc, in_maps, aux = prepare(X_train, y_train, X_test)
    res = run_bass_kernel_spmd(nc, in_maps, core_ids=list(range(NCORES)))
    return merge(res.results, aux)
